# revision 2
# baseline (speedup 1.0000x reference)
"""Stereo cost-volume kernel for Trainium2 (8 NeuronCores, SPMD).

cost[n, j, h, x] = sum_c L[n,c,h,x] * R[n,c,h,x-j], zero when x < j.

Strategy (per core, h-sharded 1/8 slab of every pyramid level):
  - PE Gram tiles with K=C=32 (bf16 inputs, fp32 PSUM):
      G[m, n] = sum_c R[c, b+m] * L[c, b+n]   (j = n - m >= 0 on the free axis)
  - PSUM -> SBUF engine copies split 96/32 partitions (DVE + ACT)
  - shear-DMA (diagonal access pattern, 768B-contiguous runs):
      s2[p, j] = G[p, p+j]  ->  s2 rows are x' = b+p, cols are pure j
  - PE transpose (via identity matmul) -> pt[j, x'] in PSUM
  - copy to SBUF, then uniform DMA-out with 512B-contiguous runs into a
    right-padded DRAM buffer dev[j, h, x = b + j + m].
  - Host slices off the x >= W pad and zero-fills the x < j triangle.
"""
import numpy as np

# (W, H, D, Hc) per pyramid level; Hc = per-core h-slab with 8 cores
LEVELS = [(512, 256, 192, 32), (256, 128, 96, 16), (128, 64, 48, 8)]
N_CORES = 8
C = 32


def _build_program():
    import concourse.bacc as bacc
    import concourse.mybir as mybir
    import concourse.bass as bass
    from concourse.tile import TileContext

    f32 = mybir.dt.float32
    bf16 = mybir.dt.bfloat16

    nc = bacc.Bacc("TRN2", target_bir_lowering=False, debug=False)

    ins = {}
    outs = {}
    for li, (W, H, D, Hc) in enumerate(LEVELS):
        ins[f"lp{li}"] = nc.dram_tensor(f"lp{li}", [C, Hc, W + D], f32, kind="ExternalInput")
        ins[f"r{li}"] = nc.dram_tensor(f"r{li}", [C, Hc, W], f32, kind="ExternalInput")
        Wp = W + D - 1
        outs[f"dev{li}"] = nc.dram_tensor(f"dev{li}", [D, Hc, Wp], f32, kind="ExternalOutput")
    idm = nc.dram_tensor("idm", [128, 128], f32, kind="ExternalInput")

    with TileContext(nc) as tc:
        with tc.tile_pool(name="const", bufs=1) as cpool:
            ident = cpool.tile([128, 128], f32)
            nc.sync.dma_start(ident[:], idm[:])

            for li, (W, H, D, Hc) in enumerate(LEVELS):
                NW = 128 + D
                Wp = W + D - 1
                lp, r, dev = ins[f"lp{li}"], ins[f"r{li}"], outs[f"dev{li}"]
                with (
                    tc.tile_pool(name=f"slab{li}", bufs=1) as spool,
                    tc.tile_pool(name=f"work{li}", bufs=3) as wpool,
                    tc.tile_pool(name=f"gpsum{li}", bufs=3, space="PSUM") as gpool,
                    tc.tile_pool(name=f"tpsum{li}", bufs=3, space="PSUM") as tpool,
                ):
                    lt = spool.tile([C, Hc, W + D], bf16)
                    rt = spool.tile([C, Hc, W], bf16)
                    nc.gpsimd.dma_start(lt[:], lp[:])   # SWDGE cast f32 -> bf16
                    nc.gpsimd.dma_start(rt[:], r[:])

                    # j-chunks of <= 96 for the transpose stage
                    jchunks = []
                    j0 = 0
                    while j0 < D:
                        jchunks.append((j0, min(96, D - j0)))
                        j0 += 96

                    for h in range(Hc):
                        for b in range(0, W, 128):
                            # Gram matmul: G[m, n] = sum_c R[c,h,b+m] L[c,h,b+n]
                            g = gpool.tile([128, NW], mybir.dt.float32, tag="g")
                            nc.tensor.matmul(
                                g[:],
                                rt[:, h, b:b + 128],
                                lt[:, h, b:b + NW],
                                start=True, stop=True,
                            )

                            # PSUM -> SBUF split 96/32 (legal diag-DMA P + 2 engines)
                            ga = wpool.tile([96, NW], mybir.dt.float32, tag="ga")
                            gb = wpool.tile([32, NW], mybir.dt.float32, tag="gb")
                            nc.vector.tensor_copy(ga[:], g[0:96, :])
                            nc.scalar.copy(gb[:], g[96:128, :])

                            # shear: s2[p, j] = G[p, p + j]
                            s2 = wpool.tile([128, D], mybir.dt.float32, tag="s2")
                            nc.sync.dma_start(
                                s2[0:96, :],
                                bass.AP(ga.tensor, ga.offset, [[NW + 1, 96], [1, D]]),
                            )
                            nc.sync.dma_start(
                                s2[96:128, :],
                                bass.AP(gb.tensor, gb.offset + 96, [[NW + 1, 32], [1, D]]),
                            )

                            # transpose chunks + copy out + DMA out
                            for ci, (j0, jn) in enumerate(jchunks):
                                pt = tpool.tile([96, 128], mybir.dt.float32, tag="pt")
                                nc.tensor.transpose(
                                    pt[0:jn, :],
                                    s2[:, j0:j0 + jn],
                                    ident[:],
                                )
                                st = wpool.tile([96, 128], mybir.dt.float32, tag=f"st{ci}")
                                if ci % 2 == 0:
                                    nc.vector.tensor_copy(st[0:jn, :], pt[0:jn, :])
                                else:
                                    nc.scalar.copy(st[0:jn, :], pt[0:jn, :])
                                # dev[j0+jj, h, b + j0 + jj + m] for jj<jn, m<128
                                dst_off = (j0 * Hc + h) * Wp + b + j0
                                nc.scalar.dma_start(
                                    bass.AP(dev.ap().tensor, dst_off,
                                            [[Hc * Wp + 1, jn], [1, 128]]),
                                    st[0:jn, :],
                                )
    nc.compile()
    return nc


_PROGRAM_CACHE = {}


def _get_program():
    if "nc" not in _PROGRAM_CACHE:
        _PROGRAM_CACHE["nc"] = _build_program()
    return _PROGRAM_CACHE["nc"]


def kernel(left0, right0, left1, right1, left2, right2, _trace=False):
    from concourse.bass_utils import run_bass_kernel_spmd

    lefts = [left0, left1, left2]
    rights = [right0, right1, right2]
    nc = _get_program()

    idm = np.eye(128, dtype=np.float32)
    in_maps = []
    for k in range(N_CORES):
        m = {"idm": idm}
        for li, (W, H, D, Hc) in enumerate(LEVELS):
            Ls = np.asarray(lefts[li])[0, :, k * Hc:(k + 1) * Hc, :]
            Rs = np.asarray(rights[li])[0, :, k * Hc:(k + 1) * Hc, :]
            m[f"lp{li}"] = np.ascontiguousarray(
                np.pad(Ls, ((0, 0), (0, 0), (0, D))), dtype=np.float32)
            m[f"r{li}"] = np.ascontiguousarray(Rs, dtype=np.float32)
        in_maps.append(m)

    kw = {}
    if _trace:
        kw = dict(trace=True, trace_cores=[0])
    results = run_bass_kernel_spmd(nc, in_maps, core_ids=list(range(N_CORES)), **kw)

    out = []
    for li, (W, H, D, Hc) in enumerate(LEVELS):
        vol = np.zeros((1, D, H, W), dtype=np.float32)
        jj = np.arange(D)[:, None, None]
        xx = np.arange(W)[None, None, :]
        band = xx >= jj
        for k in range(N_CORES):
            dv = results.results[k][f"dev{li}"][:, :, :W]
            vol[0, :, k * Hc:(k + 1) * Hc, :] = np.where(band, dv, 0.0)
        out.append(vol)
    if _trace:
        return tuple(out), results
    return tuple(out)


# revision 5
# speedup vs baseline: 1.2054x; 1.2054x over previous
"""Stereo cost-volume kernel for Trainium2 (8 NeuronCores, SPMD).

cost[n, j, h, x] = sum_c L[n,c,h,x] * R[n,c,h,x-j], zero when x < j.

Per core (h-sharded 1/8 slab of every pyramid level), batches of 2 blocks:
  - PE Gram tiles, K=C=32, bf16 inputs, fp32 PSUM (bank-aligned pairs):
      G[m, n] = sum_c R[c, b+m] * L[c, b+n]   (j = n - m >= 0 on free axis)
  - PSUM -> SBUF cast-copies to bf16, split 96/32 partitions (DVE + ACT)
  - shear-DMA (diagonal AP): s2[p, t, j] = G_t[p, p+j] -> pure-j columns
  - PE transpose (identity matmul, bf16) -> pt[j, x'] PSUM
  - copy to SBUF, uniform DMA-out (256B runs) into right-padded bf16 DRAM
    dev[j, h, x = b + j + m]; host upcasts, slices pad, zero-fills x < j.
"""
import numpy as np

# (W, H, D, Hc) per pyramid level; Hc = per-core h-slab with 8 cores
LEVELS = [(512, 256, 192, 32), (256, 128, 96, 16), (128, 64, 48, 8)]
N_CORES = 8
C = 32


def _build_program():
    import concourse.bacc as bacc
    import concourse.mybir as mybir
    import concourse.bass as bass
    from concourse.tile import TileContext

    f32 = mybir.dt.float32
    bf16 = mybir.dt.bfloat16

    nc = bacc.Bacc("TRN2", target_bir_lowering=False, debug=False)

    ins = {}
    outs = {}
    for li, (W, H, D, Hc) in enumerate(LEVELS):
        ins[f"lp{li}"] = nc.dram_tensor(f"lp{li}", [C, Hc, W + D], f32, kind="ExternalInput")
        ins[f"r{li}"] = nc.dram_tensor(f"r{li}", [C, Hc, W], f32, kind="ExternalInput")
        Wp = W + D - 1
        outs[f"dev{li}"] = nc.dram_tensor(f"dev{li}", [D, Hc, Wp], bf16, kind="ExternalOutput")
    idm = nc.dram_tensor("idm", [128, 128], f32, kind="ExternalInput")

    with TileContext(nc) as tc:
        with tc.tile_pool(name="const", bufs=1) as cpool:
            identb = cpool.tile([128, 128], bf16)
            nc.gpsimd.dma_start(identb[:], idm[:])  # SWDGE cast f32->bf16

            for li, (W, H, D, Hc) in enumerate(LEVELS):
                NW = 128 + D
                Wp = W + D - 1
                lp, r, dev = ins[f"lp{li}"], ins[f"r{li}"], outs[f"dev{li}"]
                devt = dev.ap().tensor

                # batch of 2 (h, b) blocks per PSUM tile / shear / out group
                blocks = [(h, b) for h in range(Hc) for b in range(0, W, 128)]
                pairs = [(blocks[i], blocks[i + 1]) for i in range(0, len(blocks), 2)]
                # j-chunks of <= 96 for the transpose stage
                jchunks = []
                j0 = 0
                while j0 < D:
                    jchunks.append((j0, min(96, D - j0)))
                    j0 += 96
                nch = len(jchunks)

                with (
                    tc.tile_pool(name=f"slab{li}", bufs=1) as spool,
                    tc.tile_pool(name=f"work{li}", bufs=3) as wpool,
                    tc.tile_pool(name=f"gpsum{li}", bufs=2, space="PSUM") as gpool,
                    tc.tile_pool(name=f"tpsum{li}", bufs=3, space="PSUM") as tpool,
                ):
                    lt = spool.tile([C, Hc, W + D], bf16)
                    rt = spool.tile([C, Hc, W], bf16)
                    nc.gpsimd.dma_start(lt[:], lp[:])   # SWDGE cast f32 -> bf16
                    nc.gpsimd.dma_start(rt[:], r[:])

                    for (h0, b0), (h1, b1) in pairs:
                        hb = ((h0, b0), (h1, b1))
                        # two Gram matmuls into one 2-bank PSUM tile
                        g = gpool.tile([128, 1024], f32, tag="g")
                        for t, (h, b) in enumerate(hb):
                            nc.tensor.matmul(
                                g[:, 512 * t:512 * t + NW],
                                rt[:, h, b:b + 128],
                                lt[:, h, b:b + NW],
                                start=True, stop=True,
                            )

                        # PSUM -> SBUF bf16 cast-copies, split 96/32
                        ga = wpool.tile([96, 2, NW], bf16, tag="ga")
                        gb = wpool.tile([32, 2, NW], bf16, tag="gb")
                        gsrc = bass.AP(g.tensor, g.offset, [[g.ap[0][0], 128], [512, 2], [1, NW]])
                        nc.vector.tensor_copy(ga[:], gsrc[0:96])
                        nc.scalar.copy(gb[:], gsrc[96:128])

                        # shear: s2[p, t, j] = G_t[p, p + j]
                        s2 = wpool.tile([128, 2, D], bf16, tag="s2")
                        nc.sync.dma_start(
                            bass.AP(s2.tensor, s2.offset, [[2 * D, 96], [D, 2], [1, D]]),
                            bass.AP(ga.tensor, ga.offset, [[2 * NW + 1, 96], [NW, 2], [1, D]]),
                        )
                        nc.sync.dma_start(
                            bass.AP(s2.tensor, s2.offset + 96 * 2 * D, [[2 * D, 32], [D, 2], [1, D]]),
                            bass.AP(gb.tensor, gb.offset + 96, [[2 * NW + 1, 32], [NW, 2], [1, D]]),
                        )

                        # transposes: pt[jj, (t, ci), m] for each block/chunk
                        pt = tpool.tile([96, 2 * nch, 128], bf16, tag="pt")
                        for t in range(2):
                            for ci, (j0, jn) in enumerate(jchunks):
                                nc.tensor.transpose(
                                    pt[0:jn, 2 * ci + t, :],
                                    s2[:, t, j0:j0 + jn],
                                    identb[:],
                                )
                        st = wpool.tile([96, 2 * nch, 128], bf16, tag="st")
                        nc.vector.tensor_copy(st[:], pt[:])

                        # out-DMA per chunk: dev[j0+jj, h_t, b_t + j0 + jj + m]
                        for ci, (j0, jn) in enumerate(jchunks):
                            o0 = (j0 * Hc + h0) * Wp + b0 + j0
                            o1 = (j0 * Hc + h1) * Wp + b1 + j0
                            nc.scalar.dma_start(
                                bass.AP(devt, o0, [[Hc * Wp + 1, jn], [1, 128]]),
                                st[0:jn, 2 * ci + 0, :],
                            )
                            nc.scalar.dma_start(
                                bass.AP(devt, o1, [[Hc * Wp + 1, jn], [1, 128]]),
                                st[0:jn, 2 * ci + 1, :],
                            )
    nc.compile()
    return nc


_PROGRAM_CACHE = {}


def _get_program():
    if "nc" not in _PROGRAM_CACHE:
        _PROGRAM_CACHE["nc"] = _build_program()
    return _PROGRAM_CACHE["nc"]


def kernel(left0, right0, left1, right1, left2, right2, _trace=False):
    from concourse.bass_utils import run_bass_kernel_spmd

    lefts = [left0, left1, left2]
    rights = [right0, right1, right2]
    nc = _get_program()

    idm = np.eye(128, dtype=np.float32)
    in_maps = []
    for k in range(N_CORES):
        m = {"idm": idm}
        for li, (W, H, D, Hc) in enumerate(LEVELS):
            Ls = np.asarray(lefts[li])[0, :, k * Hc:(k + 1) * Hc, :]
            Rs = np.asarray(rights[li])[0, :, k * Hc:(k + 1) * Hc, :]
            m[f"lp{li}"] = np.ascontiguousarray(
                np.pad(Ls, ((0, 0), (0, 0), (0, D))), dtype=np.float32)
            m[f"r{li}"] = np.ascontiguousarray(Rs, dtype=np.float32)
        in_maps.append(m)

    kw = {}
    if _trace:
        kw = dict(trace=True, trace_cores=[0])
    results = run_bass_kernel_spmd(nc, in_maps, core_ids=list(range(N_CORES)), **kw)

    out = []
    for li, (W, H, D, Hc) in enumerate(LEVELS):
        vol = np.zeros((1, D, H, W), dtype=np.float32)
        jj = np.arange(D)[:, None, None]
        xx = np.arange(W)[None, None, :]
        band = xx >= jj
        for k in range(N_CORES):
            dv = np.asarray(results.results[k][f"dev{li}"][:, :, :W], dtype=np.float32)
            vol[0, :, k * Hc:(k + 1) * Hc, :] = np.where(band, dv, 0.0)
        out.append(vol)
    if _trace:
        return tuple(out), results
    return tuple(out)


# revision 6
# speedup vs baseline: 1.7284x; 1.4339x over previous
"""Stereo cost-volume kernel for Trainium2 (8 NeuronCores, SPMD).

cost[n, j, h, x] = sum_c L[n,c,h,x] * R[n,c,h,x-j], zero when x < j.

Per core (h-sharded 1/8 slab of every pyramid level), batches of 2 blocks:
  - PE Gram tiles, K=C=32, bf16 inputs, fp32 PSUM (bank-aligned pairs):
      G[m, n] = sum_c R[c, b+m] * L[c, b+n]   (j = n - m >= 0 on free axis)
  - PSUM -> SBUF cast-copies to bf16, split 96/32 partitions (DVE + ACT)
  - shear-DMA (diagonal AP): s2[p, t, j] = G_t[p, p+j] -> pure-j columns
  - PE transpose (identity matmul, bf16) -> pt[j, x'] PSUM
  - copy to SBUF, uniform DMA-out (256B runs) into right-padded bf16 DRAM
    dev[j, h, x = b + j + m]; host upcasts, slices pad, zero-fills x < j.
"""
import numpy as np

# (W, H, D, Hc) per pyramid level; Hc = per-core h-slab with 8 cores
LEVELS = [(512, 256, 192, 32), (256, 128, 96, 16), (128, 64, 48, 8)]
N_CORES = 8
C = 32


def _build_program():
    import concourse.bacc as bacc
    import concourse.mybir as mybir
    import concourse.bass as bass
    from concourse.tile import TileContext

    f32 = mybir.dt.float32
    bf16 = mybir.dt.bfloat16

    nc = bacc.Bacc("TRN2", target_bir_lowering=False, debug=False)

    ins = {}
    outs = {}
    for li, (W, H, D, Hc) in enumerate(LEVELS):
        ins[f"lp{li}"] = nc.dram_tensor(f"lp{li}", [C, Hc, W + D], f32, kind="ExternalInput")
        ins[f"r{li}"] = nc.dram_tensor(f"r{li}", [C, Hc, W], f32, kind="ExternalInput")
        Wp = W + D - 1
        outs[f"dev{li}"] = nc.dram_tensor(f"dev{li}", [D, Hc, Wp], bf16, kind="ExternalOutput")
    idm = nc.dram_tensor("idm", [128, 128], f32, kind="ExternalInput")

    with TileContext(nc) as tc:
        with tc.tile_pool(name="const", bufs=1) as cpool:
            identb = cpool.tile([128, 128], bf16)
            nc.gpsimd.dma_start(identb[:], idm[:])  # SWDGE cast f32->bf16

            for li, (W, H, D, Hc) in enumerate(LEVELS):
                NW = 128 + D
                Wp = W + D - 1
                lp, r, dev = ins[f"lp{li}"], ins[f"r{li}"], outs[f"dev{li}"]
                devt = dev.ap().tensor

                # batch of 2 (h, b) blocks per PSUM tile / shear / out group
                blocks = [(h, b) for h in range(Hc) for b in range(0, W, 128)]
                pairs = [(blocks[i], blocks[i + 1]) for i in range(0, len(blocks), 2)]
                # j-chunks of <= 96 for the transpose stage
                jchunks = []
                j0 = 0
                while j0 < D:
                    jchunks.append((j0, min(96, D - j0)))
                    j0 += 96
                nch = len(jchunks)

                with (
                    tc.tile_pool(name=f"slab{li}", bufs=1) as spool,
                    tc.tile_pool(name=f"work{li}", bufs=6) as wpool,
                    tc.tile_pool(name=f"gpsum{li}", bufs=3, space="PSUM") as gpool,
                    tc.tile_pool(name=f"tpsum{li}", bufs=2, space="PSUM") as tpool,
                ):
                    lt = spool.tile([C, Hc, W + D], bf16)
                    rt = spool.tile([C, Hc, W], bf16)
                    nc.gpsimd.dma_start(lt[:], lp[:])   # SWDGE cast f32 -> bf16
                    nc.gpsimd.dma_start(rt[:], r[:])

                    pi = 0
                    for (h0, b0), (h1, b1) in pairs:
                        pi += 1
                        hb = ((h0, b0), (h1, b1))
                        # two Gram matmuls into one 2-bank PSUM tile
                        g = gpool.tile([128, 1024], f32, tag="g")
                        for t, (h, b) in enumerate(hb):
                            nc.tensor.matmul(
                                g[:, 512 * t:512 * t + NW],
                                rt[:, h, b:b + 128],
                                lt[:, h, b:b + NW],
                                start=True, stop=True,
                            )

                        # PSUM -> SBUF bf16 cast-copies, split 96/32
                        ga = wpool.tile([96, 2, NW], bf16, tag="ga")
                        gb = wpool.tile([32, 2, NW], bf16, tag="gb")
                        gsrc = bass.AP(g.tensor, g.offset, [[g.ap[0][0], 128], [512, 2], [1, NW]])
                        nc.vector.tensor_copy(ga[:], gsrc[0:96])
                        nc.scalar.copy(gb[:], gsrc[96:128])

                        # shear: s2[p, t, j] = G_t[p, p + j]
                        s2 = wpool.tile([128, 2, D], bf16, tag="s2")
                        nc.sync.dma_start(
                            bass.AP(s2.tensor, s2.offset, [[2 * D, 96], [D, 2], [1, D]]),
                            bass.AP(ga.tensor, ga.offset, [[2 * NW + 1, 96], [NW, 2], [1, D]]),
                        )
                        nc.sync.dma_start(
                            bass.AP(s2.tensor, s2.offset + 96 * 2 * D, [[2 * D, 32], [D, 2], [1, D]]),
                            bass.AP(gb.tensor, gb.offset + 96, [[2 * NW + 1, 32], [NW, 2], [1, D]]),
                        )

                        # transposes: pt[jj, (t, ci), m] for each block/chunk
                        pt = tpool.tile([96, 2 * nch, 128], bf16, tag="pt")
                        for t in range(2):
                            for ci, (j0, jn) in enumerate(jchunks):
                                nc.tensor.transpose(
                                    pt[0:jn, 2 * ci + t, :],
                                    s2[:, t, j0:j0 + jn],
                                    identb[:],
                                )
                        st = wpool.tile([96, 2 * nch, 128], bf16, tag="st")
                        if pi % 2 == 0:
                            nc.vector.tensor_copy(st[:], pt[:])
                        else:
                            nc.scalar.copy(st[:], pt[:])

                        # merged out-DMA per chunk (pair blocks are adjacent)
                        for ci, (j0, jn) in enumerate(jchunks):
                            o0 = (j0 * Hc + h0) * Wp + b0 + j0
                            if h1 == h0 and b1 == b0 + 128:
                                dst = bass.AP(devt, o0, [[Hc * Wp + 1, jn], [1, 256]])
                            elif b1 == b0 and h1 == h0 + 1:
                                dst = bass.AP(devt, o0, [[Hc * Wp + 1, jn], [Wp, 2], [1, 128]])
                            else:
                                dst = None
                            if dst is not None:
                                nc.scalar.dma_start(
                                    dst,
                                    bass.AP(st.tensor, st.offset + 2 * ci * 128,
                                            [[2 * nch * 128, jn], [1, 256]]),
                                )
                            else:
                                o1 = (j0 * Hc + h1) * Wp + b1 + j0
                                nc.scalar.dma_start(
                                    bass.AP(devt, o0, [[Hc * Wp + 1, jn], [1, 128]]),
                                    st[0:jn, 2 * ci + 0, :],
                                )
                                nc.scalar.dma_start(
                                    bass.AP(devt, o1, [[Hc * Wp + 1, jn], [1, 128]]),
                                    st[0:jn, 2 * ci + 1, :],
                                )
    nc.compile()
    return nc


_PROGRAM_CACHE = {}


def _get_program():
    if "nc" not in _PROGRAM_CACHE:
        _PROGRAM_CACHE["nc"] = _build_program()
    return _PROGRAM_CACHE["nc"]


def kernel(left0, right0, left1, right1, left2, right2, _trace=False):
    from concourse.bass_utils import run_bass_kernel_spmd

    lefts = [left0, left1, left2]
    rights = [right0, right1, right2]
    nc = _get_program()

    idm = np.eye(128, dtype=np.float32)
    in_maps = []
    for k in range(N_CORES):
        m = {"idm": idm}
        for li, (W, H, D, Hc) in enumerate(LEVELS):
            Ls = np.asarray(lefts[li])[0, :, k * Hc:(k + 1) * Hc, :]
            Rs = np.asarray(rights[li])[0, :, k * Hc:(k + 1) * Hc, :]
            m[f"lp{li}"] = np.ascontiguousarray(
                np.pad(Ls, ((0, 0), (0, 0), (0, D))), dtype=np.float32)
            m[f"r{li}"] = np.ascontiguousarray(Rs, dtype=np.float32)
        in_maps.append(m)

    kw = {}
    if _trace:
        kw = dict(trace=True, trace_cores=[0])
    results = run_bass_kernel_spmd(nc, in_maps, core_ids=list(range(N_CORES)), **kw)

    out = []
    for li, (W, H, D, Hc) in enumerate(LEVELS):
        vol = np.zeros((1, D, H, W), dtype=np.float32)
        jj = np.arange(D)[:, None, None]
        xx = np.arange(W)[None, None, :]
        band = xx >= jj
        for k in range(N_CORES):
            dv = np.asarray(results.results[k][f"dev{li}"][:, :, :W], dtype=np.float32)
            vol[0, :, k * Hc:(k + 1) * Hc, :] = np.where(band, dv, 0.0)
        out.append(vol)
    if _trace:
        return tuple(out), results
    return tuple(out)


# revision 7
# speedup vs baseline: 2.0104x; 1.1632x over previous
"""Stereo cost-volume kernel for Trainium2 (8 NeuronCores, SPMD).

cost[n, j, h, x] = sum_c L[n,c,h,x] * R[n,c,h,x-j], zero when x < j.

Per core (h-sharded 1/8 slab of every pyramid level), batches of 2 blocks:
  - PE Gram tiles, K=C=32, bf16 inputs, fp32 PSUM (bank-aligned pairs):
      G[m, n] = sum_c R[c, b+m] * L[c, b+n]   (j = n - m >= 0 on free axis)
  - PSUM -> SBUF cast-copies to bf16, split 96/32 partitions (DVE + ACT)
  - shear-DMA (diagonal AP): s2[p, t, j] = G_t[p, p+j] -> pure-j columns
  - PE transpose (identity matmul, bf16) -> pt[j, x'] PSUM
  - copy to SBUF, merged DMA-out (512B runs) into right-padded bf16 DRAM
    dev[j, h, x = b + j + m]; host upcasts, slices pad, zero-fills x < j.
Emission is software-pipelined (stage-staggered) so the Tile scheduler
overlaps ~6 pairs in flight.
"""
import numpy as np

# (W, H, D, Hc) per pyramid level; Hc = per-core h-slab with 8 cores
LEVELS = [(512, 256, 192, 32), (256, 128, 96, 16), (128, 64, 48, 8)]
N_CORES = 8
C = 32


def _build_program():
    import concourse.bacc as bacc
    import concourse.mybir as mybir
    import concourse.bass as bass
    from concourse.tile import TileContext

    f32 = mybir.dt.float32
    bf16 = mybir.dt.bfloat16

    nc = bacc.Bacc("TRN2", target_bir_lowering=False, debug=False)

    ins = {}
    outs = {}
    for li, (W, H, D, Hc) in enumerate(LEVELS):
        ins[f"lp{li}"] = nc.dram_tensor(f"lp{li}", [C, Hc, W + D], f32, kind="ExternalInput")
        ins[f"r{li}"] = nc.dram_tensor(f"r{li}", [C, Hc, W], f32, kind="ExternalInput")
        Wp = W + D - 1
        outs[f"dev{li}"] = nc.dram_tensor(f"dev{li}", [D, Hc, Wp], bf16, kind="ExternalOutput")
    idm = nc.dram_tensor("idm", [128, 128], f32, kind="ExternalInput")

    with TileContext(nc) as tc:
        with tc.tile_pool(name="const", bufs=1) as cpool:
            identb = cpool.tile([128, 128], bf16)
            nc.gpsimd.dma_start(identb[:], idm[:])  # SWDGE cast f32->bf16

            for li, (W, H, D, Hc) in enumerate(LEVELS):
                NW = 128 + D
                Wp = W + D - 1
                lp, r, dev = ins[f"lp{li}"], ins[f"r{li}"], outs[f"dev{li}"]
                devt = dev.ap().tensor

                blocks = [(h, b) for h in range(Hc) for b in range(0, W, 128)]
                pairs = [(blocks[i], blocks[i + 1]) for i in range(0, len(blocks), 2)]
                jchunks = []
                j0 = 0
                while j0 < D:
                    jchunks.append((j0, min(96, D - j0)))
                    j0 += 96
                nch = len(jchunks)

                with (
                    tc.tile_pool(name=f"slab{li}", bufs=1) as spool,
                    tc.tile_pool(name=f"work{li}", bufs=8) as wpool,
                    tc.tile_pool(name=f"gpsum{li}", bufs=2, space="PSUM") as gpool,
                    tc.tile_pool(name=f"tpsum{li}", bufs=4, space="PSUM") as tpool,
                ):
                    lt = spool.tile([C, Hc, W + D], bf16)
                    rt = spool.tile([C, Hc, W], bf16)
                    nc.gpsimd.dma_start(lt[:], lp[:])   # SWDGE cast f32 -> bf16
                    nc.gpsimd.dma_start(rt[:], r[:])

                    # ---- software-pipelined emission ----
                    state = {}

                    def st_mm(i):
                        (h0, b0), (h1, b1) = pairs[i]
                        g = gpool.tile([128, 1024], f32, tag="g")
                        for t, (h, b) in enumerate(((h0, b0), (h1, b1))):
                            nc.tensor.matmul(
                                g[:, 512 * t:512 * t + NW],
                                rt[:, h, b:b + 128],
                                lt[:, h, b:b + NW],
                                start=True, stop=True,
                            )
                        state[(i, "g")] = g

                    def st_copy(i):
                        g = state.pop((i, "g"))
                        ga = wpool.tile([96, 2, NW], bf16, tag="ga")
                        gb = wpool.tile([32, 2, NW], bf16, tag="gb")
                        gsrc = bass.AP(g.tensor, g.offset,
                                       [[g.ap[0][0], 128], [512, 2], [1, NW]])
                        nc.vector.tensor_copy(ga[:], gsrc[0:96])
                        nc.scalar.copy(gb[:], gsrc[96:128])
                        state[(i, "gab")] = (ga, gb)

                    def st_shear(i):
                        ga, gb = state.pop((i, "gab"))
                        s2 = wpool.tile([128, 2, D], bf16, tag="s2")
                        nc.sync.dma_start(
                            bass.AP(s2.tensor, s2.offset, [[2 * D, 96], [D, 2], [1, D]]),
                            bass.AP(ga.tensor, ga.offset, [[2 * NW + 1, 96], [NW, 2], [1, D]]),
                        )
                        nc.sync.dma_start(
                            bass.AP(s2.tensor, s2.offset + 96 * 2 * D, [[2 * D, 32], [D, 2], [1, D]]),
                            bass.AP(gb.tensor, gb.offset + 96, [[2 * NW + 1, 32], [NW, 2], [1, D]]),
                        )
                        state[(i, "s2")] = s2

                    def st_tr(i):
                        s2 = state.pop((i, "s2"))
                        pt = tpool.tile([96, 2 * nch, 128], bf16, tag="pt")
                        for t in range(2):
                            for ci, (j0, jn) in enumerate(jchunks):
                                nc.tensor.transpose(
                                    pt[0:jn, 2 * ci + t, :],
                                    s2[:, t, j0:j0 + jn],
                                    identb[:],
                                )
                        state[(i, "pt")] = pt

                    def st_stcopy(i):
                        pt = state.pop((i, "pt"))
                        st = wpool.tile([96, 2 * nch, 128], bf16, tag="st")
                        if i % 2 == 0:
                            nc.vector.tensor_copy(st[:], pt[:])
                        else:
                            nc.scalar.copy(st[:], pt[:])
                        state[(i, "st")] = st

                    def st_out(i):
                        (h0, b0), (h1, b1) = pairs[i]
                        st = state.pop((i, "st"))
                        for ci, (j0, jn) in enumerate(jchunks):
                            o0 = (j0 * Hc + h0) * Wp + b0 + j0
                            if h1 == h0 and b1 == b0 + 128:
                                dst = bass.AP(devt, o0, [[Hc * Wp + 1, jn], [1, 256]])
                            elif b1 == b0 and h1 == h0 + 1:
                                dst = bass.AP(devt, o0, [[Hc * Wp + 1, jn], [Wp, 2], [1, 128]])
                            else:
                                dst = None
                            if dst is not None:
                                nc.scalar.dma_start(
                                    dst,
                                    bass.AP(st.tensor, st.offset + 2 * ci * 128,
                                            [[2 * nch * 128, jn], [1, 256]]),
                                )
                            else:
                                o1 = (j0 * Hc + h1) * Wp + b1 + j0
                                nc.scalar.dma_start(
                                    bass.AP(devt, o0, [[Hc * Wp + 1, jn], [1, 128]]),
                                    st[0:jn, 2 * ci + 0, :],
                                )
                                nc.scalar.dma_start(
                                    bass.AP(devt, o1, [[Hc * Wp + 1, jn], [1, 128]]),
                                    st[0:jn, 2 * ci + 1, :],
                                )

                    stages = [st_mm, st_copy, st_shear, st_tr, st_stcopy, st_out]
                    ns = len(stages)
                    npairs = len(pairs)
                    for step in range(npairs + ns - 1):
                        for s in range(ns):
                            i = step - s
                            if 0 <= i < npairs:
                                stages[s](i)
    nc.compile()
    return nc


_PROGRAM_CACHE = {}


def _get_program():
    if "nc" not in _PROGRAM_CACHE:
        _PROGRAM_CACHE["nc"] = _build_program()
    return _PROGRAM_CACHE["nc"]


def kernel(left0, right0, left1, right1, left2, right2, _trace=False):
    from concourse.bass_utils import run_bass_kernel_spmd

    lefts = [left0, left1, left2]
    rights = [right0, right1, right2]
    nc = _get_program()

    idm = np.eye(128, dtype=np.float32)
    in_maps = []
    for k in range(N_CORES):
        m = {"idm": idm}
        for li, (W, H, D, Hc) in enumerate(LEVELS):
            Ls = np.asarray(lefts[li])[0, :, k * Hc:(k + 1) * Hc, :]
            Rs = np.asarray(rights[li])[0, :, k * Hc:(k + 1) * Hc, :]
            m[f"lp{li}"] = np.ascontiguousarray(
                np.pad(Ls, ((0, 0), (0, 0), (0, D))), dtype=np.float32)
            m[f"r{li}"] = np.ascontiguousarray(Rs, dtype=np.float32)
        in_maps.append(m)

    kw = {}
    if _trace:
        kw = dict(trace=True, trace_cores=[0])
    results = run_bass_kernel_spmd(nc, in_maps, core_ids=list(range(N_CORES)), **kw)

    out = []
    for li, (W, H, D, Hc) in enumerate(LEVELS):
        vol = np.zeros((1, D, H, W), dtype=np.float32)
        jj = np.arange(D)[:, None, None]
        xx = np.arange(W)[None, None, :]
        band = xx >= jj
        for k in range(N_CORES):
            dv = np.asarray(results.results[k][f"dev{li}"][:, :, :W], dtype=np.float32)
            vol[0, :, k * Hc:(k + 1) * Hc, :] = np.where(band, dv, 0.0)
        out.append(vol)
    if _trace:
        return tuple(out), results
    return tuple(out)


# revision 8
# speedup vs baseline: 2.0541x; 1.0217x over previous
"""Stereo cost-volume kernel for Trainium2 (8 NeuronCores, SPMD).

cost[n, j, h, x] = sum_c L[n,c,h,x] * R[n,c,h,x-j], zero when x < j.

Per core (h-sharded 1/8 slab of every pyramid level), batches of 2 blocks:
  - PE Gram tiles, K=C=32, bf16 inputs, fp32 PSUM (bank-aligned pairs):
      G[m, n] = sum_c R[c, b+m] * L[c, b+n]   (j = n - m >= 0 on free axis)
  - PSUM -> SBUF cast-copies to bf16, split 96/32 partitions (DVE + ACT)
  - shear-DMA (diagonal AP): s2[p, t, j] = G_t[p, p+j] -> pure-j columns
  - PE transpose (identity matmul, bf16) -> pt[j, x'] PSUM
  - copy to SBUF, merged DMA-out (512B runs) into right-padded bf16 DRAM
    dev[j, h, x = b + j + m]; host upcasts, slices pad, zero-fills x < j.
Emission is software-pipelined (stage-staggered) so the Tile scheduler
overlaps ~6 pairs in flight.
"""
import numpy as np

# (W, H, D, Hc) per pyramid level; Hc = per-core h-slab with 8 cores
LEVELS = [(512, 256, 192, 32), (256, 128, 96, 16), (128, 64, 48, 8)]
N_CORES = 8
C = 32


def _build_program():
    import concourse.bacc as bacc
    import concourse.mybir as mybir
    import concourse.bass as bass
    from concourse.tile import TileContext

    f32 = mybir.dt.float32
    bf16 = mybir.dt.bfloat16

    nc = bacc.Bacc("TRN2", target_bir_lowering=False, debug=False)

    ins = {}
    outs = {}
    for li, (W, H, D, Hc) in enumerate(LEVELS):
        ins[f"lp{li}"] = nc.dram_tensor(f"lp{li}", [C, Hc, W + D], f32, kind="ExternalInput")
        ins[f"r{li}"] = nc.dram_tensor(f"r{li}", [C, Hc, W], f32, kind="ExternalInput")
        Wp = W + D - 1
        outs[f"dev{li}"] = nc.dram_tensor(f"dev{li}", [D, Hc, Wp], bf16, kind="ExternalOutput")
    idm = nc.dram_tensor("idm", [128, 128], f32, kind="ExternalInput")

    with TileContext(nc) as tc:
        with tc.tile_pool(name="const", bufs=1) as cpool:
            identb = cpool.tile([128, 128], bf16)
            nc.gpsimd.dma_start(identb[:], idm[:])  # SWDGE cast f32->bf16

            for li, (W, H, D, Hc) in enumerate(LEVELS):
                NW = 128 + D
                Wp = W + D - 1
                lp, r, dev = ins[f"lp{li}"], ins[f"r{li}"], outs[f"dev{li}"]
                devt = dev.ap().tensor

                blocks = [(h, b) for h in range(Hc) for b in range(0, W, 128)]
                pairs = [(blocks[i], blocks[i + 1]) for i in range(0, len(blocks), 2)]
                jchunks = []
                j0 = 0
                while j0 < D:
                    jchunks.append((j0, min(96, D - j0)))
                    j0 += 96
                nch = len(jchunks)

                with (
                    tc.tile_pool(name=f"slab{li}", bufs=1) as spool,
                    tc.tile_pool(name=f"work{li}", bufs=8) as wpool,
                    tc.tile_pool(name=f"gpsum{li}", bufs=2, space="PSUM") as gpool,
                    tc.tile_pool(name=f"tpsum{li}", bufs=4, space="PSUM") as tpool,
                ):
                    lt = spool.tile([C, Hc, W + D], bf16)
                    rt = spool.tile([C, Hc, W], bf16)
                    nc.gpsimd.dma_start(lt[:], lp[:])   # SWDGE cast f32 -> bf16
                    nc.gpsimd.dma_start(rt[:], r[:])

                    # ---- software-pipelined emission ----
                    state = {}

                    def st_mm(i):
                        (h0, b0), (h1, b1) = pairs[i]
                        g = gpool.tile([128, 1024], f32, tag="g")
                        for t, (h, b) in enumerate(((h0, b0), (h1, b1))):
                            nc.tensor.matmul(
                                g[:, 512 * t:512 * t + NW],
                                rt[:, h, b:b + 128],
                                lt[:, h, b:b + NW],
                                start=True, stop=True,
                            )
                        state[(i, "g")] = g

                    def st_copy(i):
                        g = state.pop((i, "g"))
                        ga = wpool.tile([96, 2, NW], bf16, tag="ga")
                        gb = wpool.tile([32, 2, NW], bf16, tag="gb")
                        gsrc = bass.AP(g.tensor, g.offset,
                                       [[g.ap[0][0], 128], [512, 2], [1, NW]])
                        nc.vector.tensor_copy(ga[:], gsrc[0:96])
                        nc.scalar.copy(gb[:], gsrc[96:128])
                        state[(i, "gab")] = (ga, gb)

                    def st_shear(i):
                        ga, gb = state.pop((i, "gab"))
                        s2 = wpool.tile([128, 2, D], bf16, tag="s2")
                        nc.sync.dma_start(
                            bass.AP(s2.tensor, s2.offset, [[2 * D, 96], [D, 2], [1, D]]),
                            bass.AP(ga.tensor, ga.offset, [[2 * NW + 1, 96], [NW, 2], [1, D]]),
                        )
                        nc.sync.dma_start(
                            bass.AP(s2.tensor, s2.offset + 96 * 2 * D, [[2 * D, 32], [D, 2], [1, D]]),
                            bass.AP(gb.tensor, gb.offset + 96, [[2 * NW + 1, 32], [NW, 2], [1, D]]),
                        )
                        state[(i, "s2")] = s2

                    def st_tr(i):
                        s2 = state.pop((i, "s2"))
                        pt = tpool.tile([96, 2 * nch, 128], bf16, tag="pt")
                        for t in range(2):
                            for ci, (j0, jn) in enumerate(jchunks):
                                nc.tensor.transpose(
                                    pt[0:jn, 2 * ci + t, :],
                                    s2[:, t, j0:j0 + jn],
                                    identb[:],
                                )
                        state[(i, "pt")] = pt

                    def st_stcopy(i):
                        pt = state.pop((i, "pt"))
                        if i % 2 == 0:
                            st = wpool.tile([96, nch, 4, 128], bf16, tag="st")
                            state[(i // 2, "st")] = st
                        else:
                            st = state[(i // 2, "st")]
                        q = (i % 2) * 2
                        # pt layout [96, (ci, t), 128] -> st slots [ci, q + t]
                        dstap = bass.AP(st.tensor,
                                        st.offset + q * 128,
                                        [[nch * 512, 96], [512, nch], [128, 2], [1, 128]])
                        if i % 2 == 0:
                            nc.vector.tensor_copy(dstap, pt[:])
                        else:
                            nc.scalar.copy(dstap, pt[:])

                    def st_out(i):
                        # runs on odd i: flush quad (pairs i-1, i) = blocks q0..q3
                        if i % 2 == 0:
                            if i != npairs - 1:
                                return
                            # odd tail never happens (all level block counts
                            # are multiples of 4), but guard anyway
                            raise AssertionError("unpaired quad")
                        st = state.pop((i // 2, "st"))
                        quad = [*pairs[i - 1], *pairs[i]]
                        (h0, b0) = quad[0]
                        same_h = all(h == h0 for h, _ in quad) and \
                            [b for _, b in quad] == [b0 + 128 * t for t in range(4)]
                        consec_h = all(b == b0 for _, b in quad) and \
                            [h for h, _ in quad] == [h0 + t for t in range(4)]
                        for ci, (j0, jn) in enumerate(jchunks):
                            o0 = (j0 * Hc + h0) * Wp + b0 + j0
                            srcap = bass.AP(st.tensor, st.offset + ci * 512,
                                            [[nch * 512, jn], [1, 512]])
                            if same_h:
                                dst = bass.AP(devt, o0, [[Hc * Wp + 1, jn], [1, 512]])
                            elif consec_h:
                                dst = bass.AP(devt, o0,
                                              [[Hc * Wp + 1, jn], [Wp, 4], [1, 128]])
                            else:
                                # pairs are (h,b),(h,b+128) and (h+1,b),(h+1,b+128)
                                dst = bass.AP(devt, o0,
                                              [[Hc * Wp + 1, jn], [Wp, 2], [1, 256]])
                            nc.scalar.dma_start(dst, srcap)

                    stages = [st_mm, st_copy, st_shear, st_tr, st_stcopy, st_out]
                    ns = len(stages)
                    npairs = len(pairs)
                    for step in range(npairs + ns - 1):
                        for s in range(ns):
                            i = step - s
                            if 0 <= i < npairs:
                                stages[s](i)
    nc.compile()
    return nc


_PROGRAM_CACHE = {}


def _get_program():
    if "nc" not in _PROGRAM_CACHE:
        _PROGRAM_CACHE["nc"] = _build_program()
    return _PROGRAM_CACHE["nc"]


def kernel(left0, right0, left1, right1, left2, right2, _trace=False):
    from concourse.bass_utils import run_bass_kernel_spmd

    lefts = [left0, left1, left2]
    rights = [right0, right1, right2]
    nc = _get_program()

    idm = np.eye(128, dtype=np.float32)
    in_maps = []
    for k in range(N_CORES):
        m = {"idm": idm}
        for li, (W, H, D, Hc) in enumerate(LEVELS):
            Ls = np.asarray(lefts[li])[0, :, k * Hc:(k + 1) * Hc, :]
            Rs = np.asarray(rights[li])[0, :, k * Hc:(k + 1) * Hc, :]
            m[f"lp{li}"] = np.ascontiguousarray(
                np.pad(Ls, ((0, 0), (0, 0), (0, D))), dtype=np.float32)
            m[f"r{li}"] = np.ascontiguousarray(Rs, dtype=np.float32)
        in_maps.append(m)

    kw = {}
    if _trace:
        kw = dict(trace=True, trace_cores=[0])
    results = run_bass_kernel_spmd(nc, in_maps, core_ids=list(range(N_CORES)), **kw)

    out = []
    for li, (W, H, D, Hc) in enumerate(LEVELS):
        vol = np.zeros((1, D, H, W), dtype=np.float32)
        jj = np.arange(D)[:, None, None]
        xx = np.arange(W)[None, None, :]
        band = xx >= jj
        for k in range(N_CORES):
            dv = np.asarray(results.results[k][f"dev{li}"][:, :, :W], dtype=np.float32)
            vol[0, :, k * Hc:(k + 1) * Hc, :] = np.where(band, dv, 0.0)
        out.append(vol)
    if _trace:
        return tuple(out), results
    return tuple(out)


# revision 9
# speedup vs baseline: 2.2279x; 1.0846x over previous
"""Stereo cost-volume kernel for Trainium2 (8 NeuronCores, SPMD).

cost[n, j, h, x] = sum_c L[n,c,h,x] * R[n,c,h,x-j], zero when x < j.

Per core (h-sharded 1/8 slab of every pyramid level), groups of 8 blocks:
  - PE Gram tiles, K=C=32, bf16 inputs, fp32 PSUM (bank-aligned pairs):
      G[m, n] = sum_c R[c, b+m] * L[c, b+n]   (j = n - m >= 0 on free axis)
  - PSUM -> SBUF cast-copies to bf16, split 96/32 partitions (DVE + ACT)
  - group shear-DMA (diagonal AP, T=8 blocks/op): s2[p, t, j] = G_t[p, p+j]
  - PE transpose (identity matmul, bf16) -> pt[j, x'] PSUM
  - copy to SBUF octet tile, merged DMA-out (1KB runs) into right-padded
    bf16 DRAM dev[j, h, x]; host upcasts, slices pad, zero-fills x < j.
Emission is software-pipelined at group granularity.
"""
import numpy as np

# (W, H, D, Hc) per pyramid level; Hc = per-core h-slab with 8 cores
LEVELS = [(512, 256, 192, 32), (256, 128, 96, 16), (128, 64, 48, 8)]
N_CORES = 8
C = 32
T = 8  # blocks per shear/out group


def _build_program():
    import concourse.bacc as bacc
    import concourse.mybir as mybir
    import concourse.bass as bass
    from concourse.tile import TileContext

    f32 = mybir.dt.float32
    bf16 = mybir.dt.bfloat16

    nc = bacc.Bacc("TRN2", target_bir_lowering=False, debug=False)

    ins = {}
    outs = {}
    for li, (W, H, D, Hc) in enumerate(LEVELS):
        ins[f"lp{li}"] = nc.dram_tensor(f"lp{li}", [C, Hc, W + D], f32, kind="ExternalInput")
        ins[f"r{li}"] = nc.dram_tensor(f"r{li}", [C, Hc, W], f32, kind="ExternalInput")
        Wp = W + D - 1
        outs[f"dev{li}"] = nc.dram_tensor(f"dev{li}", [D, Hc, Wp], bf16, kind="ExternalOutput")
    idm = nc.dram_tensor("idm", [128, 128], f32, kind="ExternalInput")

    with TileContext(nc) as tc:
        with tc.tile_pool(name="const", bufs=1) as cpool:
            identb = cpool.tile([128, 128], bf16)
            nc.gpsimd.dma_start(identb[:], idm[:])  # SWDGE cast f32->bf16

            for li, (W, H, D, Hc) in enumerate(LEVELS):
                NW = 128 + D
                NA = 96 + D   # ga columns (rows 0..95 read cols [p, p+D))
                NB = 128 + D - 96  # gb columns (rows 96..127 read cols [96, 128+D))
                Wp = W + D - 1
                lp, r, dev = ins[f"lp{li}"], ins[f"r{li}"], outs[f"dev{li}"]
                devt = dev.ap().tensor

                blocks = [(h, b) for h in range(Hc) for b in range(0, W, 128)]
                assert len(blocks) % T == 0
                groups = [blocks[i:i + T] for i in range(0, len(blocks), T)]
                bpw = W // 128  # blocks per h-row
                jchunks = []
                j0 = 0
                while j0 < D:
                    jchunks.append((j0, min(96, D - j0)))
                    j0 += 96
                nch = len(jchunks)

                with (
                    tc.tile_pool(name=f"slab{li}", bufs=1) as spool,
                    tc.tile_pool(name=f"work{li}", bufs=3) as wpool,
                    tc.tile_pool(name=f"gpsum{li}", bufs=2, space="PSUM") as gpool,
                    tc.tile_pool(name=f"tpsum{li}", bufs=4, space="PSUM") as tpool,
                ):
                    lt = spool.tile([C, Hc, W + D], bf16)
                    rt = spool.tile([C, Hc, W], bf16)
                    nc.gpsimd.dma_start(lt[:], lp[:])   # SWDGE cast f32 -> bf16
                    nc.gpsimd.dma_start(rt[:], r[:])

                    state = {}

                    def st_mm(gi):
                        # 4 psum pair-tiles; interleave mm + drain copies
                        ga = wpool.tile([96, T, NA], bf16, tag="ga")
                        gb = wpool.tile([32, T, NB], bf16, tag="gb")
                        state[(gi, "gab")] = (ga, gb)
                        for pi in range(T // 2):
                            g = gpool.tile([128, 1024], f32, tag="g")
                            for t in range(2):
                                h, b = groups[gi][2 * pi + t]
                                nc.tensor.matmul(
                                    g[:, 512 * t:512 * t + NW],
                                    rt[:, h, b:b + 128],
                                    lt[:, h, b:b + NW],
                                    start=True, stop=True,
                                )
                            # drain this psum pair into group tiles
                            gsa = bass.AP(g.tensor, g.offset,
                                          [[g.ap[0][0], 128], [512, 2], [1, NA]])
                            gsb = bass.AP(g.tensor, g.offset + 96,
                                          [[g.ap[0][0], 128], [512, 2], [1, NB]])
                            nc.vector.tensor_copy(
                                ga[:, 2 * pi:2 * pi + 2, :], gsa[0:96])
                            nc.scalar.copy(
                                gb[:, 2 * pi:2 * pi + 2, :], gsb[96:128])

                    def st_shear(gi):
                        ga, gb = state.pop((gi, "gab"))
                        s2 = wpool.tile([128, T, D], bf16, tag="s2")
                        # rows 0..95: col p + j  (ga holds cols [0, NA))
                        nc.sync.dma_start(
                            bass.AP(s2.tensor, s2.offset,
                                    [[T * D, 96], [D, T], [1, D]]),
                            bass.AP(ga.tensor, ga.offset,
                                    [[T * NA + 1, 96], [NA, T], [1, D]]),
                        )
                        # rows 96..127: col p + j; gb holds cols [96, 128 + D)
                        nc.sync.dma_start(
                            bass.AP(s2.tensor, s2.offset + 96 * T * D,
                                    [[T * D, 32], [D, T], [1, D]]),
                            bass.AP(gb.tensor, gb.offset,
                                    [[T * NB + 1, 32], [NB, T], [1, D]]),
                        )
                        state[(gi, "s2")] = s2

                    def st_tr(gi):
                        s2 = state.pop((gi, "s2"))
                        st = wpool.tile([96, nch, T, 128], bf16, tag="st")
                        state[(gi, "st")] = st
                        for half in range(2):
                            pt = tpool.tile([96, nch, 4, 128], bf16, tag="pt")
                            for t4 in range(4):
                                t = half * 4 + t4
                                for ci, (j0, jn) in enumerate(jchunks):
                                    nc.tensor.transpose(
                                        pt[0:jn, ci, t4, :],
                                        s2[:, t, j0:j0 + jn],
                                        identb[:],
                                    )
                            dstap = bass.AP(
                                st.tensor, st.offset + half * 4 * 128,
                                [[nch * T * 128, 96], [T * 128, nch], [128, 4], [1, 128]])
                            if half == 0:
                                nc.vector.tensor_copy(dstap, pt[:])
                            else:
                                nc.scalar.copy(dstap, pt[:])

                    def st_out(gi):
                        st = state.pop((gi, "st"))
                        grp = groups[gi]
                        (h0, b0) = grp[0]
                        nrows = T // bpw  # h-rows in this group
                        for ci, (j0, jn) in enumerate(jchunks):
                            o0 = (j0 * Hc + h0) * Wp + b0 + j0
                            srcap = bass.AP(st.tensor, st.offset + ci * T * 128,
                                            [[nch * T * 128, jn], [1, T * 128]])
                            if nrows == 1:
                                dst = bass.AP(devt, o0,
                                              [[Hc * Wp + 1, jn], [1, T * 128]])
                            else:
                                dst = bass.AP(devt, o0,
                                              [[Hc * Wp + 1, jn], [Wp, nrows],
                                               [1, (T // nrows) * 128]])
                            nc.scalar.dma_start(dst, srcap)

                    stages = [st_mm, st_shear, st_tr, st_out]
                    ns = len(stages)
                    ng = len(groups)
                    for step in range(ng + ns - 1):
                        for s in range(ns):
                            i = step - s
                            if 0 <= i < ng:
                                stages[s](i)
    nc.compile()
    return nc


_PROGRAM_CACHE = {}


def _get_program():
    if "nc" not in _PROGRAM_CACHE:
        _PROGRAM_CACHE["nc"] = _build_program()
    return _PROGRAM_CACHE["nc"]


def kernel(left0, right0, left1, right1, left2, right2, _trace=False):
    from concourse.bass_utils import run_bass_kernel_spmd

    lefts = [left0, left1, left2]
    rights = [right0, right1, right2]
    nc = _get_program()

    idm = np.eye(128, dtype=np.float32)
    in_maps = []
    for k in range(N_CORES):
        m = {"idm": idm}
        for li, (W, H, D, Hc) in enumerate(LEVELS):
            Ls = np.asarray(lefts[li])[0, :, k * Hc:(k + 1) * Hc, :]
            Rs = np.asarray(rights[li])[0, :, k * Hc:(k + 1) * Hc, :]
            m[f"lp{li}"] = np.ascontiguousarray(
                np.pad(Ls, ((0, 0), (0, 0), (0, D))), dtype=np.float32)
            m[f"r{li}"] = np.ascontiguousarray(Rs, dtype=np.float32)
        in_maps.append(m)

    kw = {}
    if _trace:
        kw = dict(trace=True, trace_cores=[0])
    results = run_bass_kernel_spmd(nc, in_maps, core_ids=list(range(N_CORES)), **kw)

    out = []
    for li, (W, H, D, Hc) in enumerate(LEVELS):
        vol = np.zeros((1, D, H, W), dtype=np.float32)
        jj = np.arange(D)[:, None, None]
        xx = np.arange(W)[None, None, :]
        band = xx >= jj
        for k in range(N_CORES):
            dv = np.asarray(results.results[k][f"dev{li}"][:, :, :W], dtype=np.float32)
            vol[0, :, k * Hc:(k + 1) * Hc, :] = np.where(band, dv, 0.0)
        out.append(vol)
    if _trace:
        return tuple(out), results
    return tuple(out)


# revision 10
# speedup vs baseline: 2.2723x; 1.0199x over previous
"""Stereo cost-volume kernel for Trainium2 (8 NeuronCores, SPMD).

cost[n, j, h, x] = sum_c L[n,c,h,x] * R[n,c,h,x-j], zero when x < j.

Per core (h-sharded 1/8 slab of every pyramid level), groups of 8 blocks:
  - PE Gram tiles, K=C=32, bf16 inputs, fp32 PSUM (bank-aligned pairs):
      G[m, n] = sum_c R[c, b+m] * L[c, b+n]   (j = n - m >= 0 on free axis)
  - PSUM -> SBUF cast-copies to bf16, split 96/32 partitions (DVE + ACT)
  - group shear-DMA (diagonal AP, T=8 blocks/op): s2[p, t, j] = G_t[p, p+j]
  - PE transpose (identity matmul, bf16) -> pt[j, x'] PSUM
  - copy to SBUF octet tile, merged DMA-out (1KB runs) into right-padded
    bf16 DRAM dev[j, h, x]; host upcasts, slices pad, zero-fills x < j.
Emission is software-pipelined at group granularity.
"""
import numpy as np

# (W, H, D, Hc) per pyramid level; Hc = per-core h-slab with 8 cores
LEVELS = [(512, 256, 192, 32), (256, 128, 96, 16), (128, 64, 48, 8)]
N_CORES = 8
C = 32
T = 8  # blocks per shear/out group


def _build_program():
    import concourse.bacc as bacc
    import concourse.mybir as mybir
    import concourse.bass as bass
    from concourse.tile import TileContext

    f32 = mybir.dt.float32
    bf16 = mybir.dt.bfloat16

    nc = bacc.Bacc("TRN2", target_bir_lowering=False, debug=False)

    ins = {}
    outs = {}
    for li, (W, H, D, Hc) in enumerate(LEVELS):
        ins[f"lp{li}"] = nc.dram_tensor(f"lp{li}", [C, Hc, W + D], f32, kind="ExternalInput")
        ins[f"r{li}"] = nc.dram_tensor(f"r{li}", [C, Hc, W], f32, kind="ExternalInput")
        Wp = W + D - 1
        outs[f"dev{li}"] = nc.dram_tensor(f"dev{li}", [D, Hc, Wp], bf16, kind="ExternalOutput")
    idm = nc.dram_tensor("idm", [128, 128], f32, kind="ExternalInput")

    with TileContext(nc) as tc:
        with tc.tile_pool(name="const", bufs=1) as cpool:
            identb = cpool.tile([128, 128], bf16)
            nc.gpsimd.dma_start(identb[:], idm[:])  # SWDGE cast f32->bf16

            for li, (W, H, D, Hc) in enumerate(LEVELS):
                NW = 128 + D
                NA = 96 + D   # ga columns (rows 0..95 read cols [p, p+D))
                NB = 128 + D - 96  # gb columns (rows 96..127 read cols [96, 128+D))
                Wp = W + D - 1
                lp, r, dev = ins[f"lp{li}"], ins[f"r{li}"], outs[f"dev{li}"]
                devt = dev.ap().tensor

                blocks = [(h, b) for h in range(Hc) for b in range(0, W, 128)]
                assert len(blocks) % T == 0
                groups = [blocks[i:i + T] for i in range(0, len(blocks), T)]
                bpw = W // 128  # blocks per h-row
                jchunks = []
                j0 = 0
                while j0 < D:
                    jchunks.append((j0, min(96, D - j0)))
                    j0 += 96
                nch = len(jchunks)

                with (
                    tc.tile_pool(name=f"slab{li}", bufs=1) as spool,
                    tc.tile_pool(name=f"work{li}", bufs=4) as wpool,
                    tc.tile_pool(name=f"gpsum{li}", bufs=2, space="PSUM") as gpool,
                    tc.tile_pool(name=f"tpsum{li}", bufs=4, space="PSUM") as tpool,
                ):
                    lt = spool.tile([C, Hc, W + D], bf16)
                    rt = spool.tile([C, Hc, W], bf16)
                    nc.gpsimd.dma_start(lt[:], lp[:])   # SWDGE cast f32 -> bf16
                    nc.gpsimd.dma_start(rt[:], r[:])

                    state = {}

                    def st_mm(gi):
                        # 4 psum pair-tiles; interleave mm + drain copies
                        ga = wpool.tile([96, T, NA], bf16, tag="ga")
                        gb = wpool.tile([32, T, NB], bf16, tag="gb")
                        state[(gi, "gab")] = (ga, gb)
                        for pi in range(T // 2):
                            g = gpool.tile([128, 1024], f32, tag="g")
                            for t in range(2):
                                h, b = groups[gi][2 * pi + t]
                                nc.tensor.matmul(
                                    g[:, 512 * t:512 * t + NW],
                                    rt[:, h, b:b + 128],
                                    lt[:, h, b:b + NW],
                                    start=True, stop=True,
                                )
                            # drain this psum pair into group tiles
                            gsa = bass.AP(g.tensor, g.offset,
                                          [[g.ap[0][0], 128], [512, 2], [1, NA]])
                            gsb = bass.AP(g.tensor, g.offset + 96,
                                          [[g.ap[0][0], 128], [512, 2], [1, NB]])
                            nc.vector.tensor_copy(
                                ga[:, 2 * pi:2 * pi + 2, :], gsa[0:96])
                            nc.scalar.copy(
                                gb[:, 2 * pi:2 * pi + 2, :], gsb[96:128])

                    def st_shear(gi):
                        ga, gb = state.pop((gi, "gab"))
                        s2 = wpool.tile([128, T, D], bf16, tag="s2")
                        # rows 0..95: col p + j  (ga holds cols [0, NA))
                        nc.sync.dma_start(
                            bass.AP(s2.tensor, s2.offset,
                                    [[T * D, 96], [D, T], [1, D]]),
                            bass.AP(ga.tensor, ga.offset,
                                    [[T * NA + 1, 96], [NA, T], [1, D]]),
                        )
                        # rows 96..127: col p + j; gb holds cols [96, 128 + D)
                        nc.sync.dma_start(
                            bass.AP(s2.tensor, s2.offset + 96 * T * D,
                                    [[T * D, 32], [D, T], [1, D]]),
                            bass.AP(gb.tensor, gb.offset,
                                    [[T * NB + 1, 32], [NB, T], [1, D]]),
                        )
                        state[(gi, "s2")] = s2

                    def st_tr(gi):
                        s2 = state.pop((gi, "s2"))
                        st = wpool.tile([96, nch, T, 128], bf16, tag="st")
                        state[(gi, "st")] = st
                        for half in range(2):
                            pt = tpool.tile([96, nch, 4, 128], bf16, tag="pt")
                            for t4 in range(4):
                                t = half * 4 + t4
                                for ci, (j0, jn) in enumerate(jchunks):
                                    nc.tensor.transpose(
                                        pt[0:jn, ci, t4, :],
                                        s2[:, t, j0:j0 + jn],
                                        identb[:],
                                    )
                            dstap = bass.AP(
                                st.tensor, st.offset + half * 4 * 128,
                                [[nch * T * 128, 96], [T * 128, nch], [128, 4], [1, 128]])
                            if half == 0:
                                nc.vector.tensor_copy(dstap, pt[:])
                            else:
                                nc.scalar.copy(dstap, pt[:])

                    def st_out(gi):
                        st = state.pop((gi, "st"))
                        grp = groups[gi]
                        (h0, b0) = grp[0]
                        nrows = T // bpw  # h-rows in this group
                        for ci, (j0, jn) in enumerate(jchunks):
                            o0 = (j0 * Hc + h0) * Wp + b0 + j0
                            srcap = bass.AP(st.tensor, st.offset + ci * T * 128,
                                            [[nch * T * 128, jn], [1, T * 128]])
                            if nrows == 1:
                                dst = bass.AP(devt, o0,
                                              [[Hc * Wp + 1, jn], [1, T * 128]])
                            else:
                                dst = bass.AP(devt, o0,
                                              [[Hc * Wp + 1, jn], [Wp, nrows],
                                               [1, (T // nrows) * 128]])
                            nc.sync.dma_start(dst, srcap)

                    stages = [st_mm, st_shear, st_tr, st_out]
                    ns = len(stages)
                    ng = len(groups)
                    for step in range(ng + ns - 1):
                        for s in range(ns):
                            i = step - s
                            if 0 <= i < ng:
                                stages[s](i)
    nc.compile()
    return nc


_PROGRAM_CACHE = {}


def _get_program():
    if "nc" not in _PROGRAM_CACHE:
        _PROGRAM_CACHE["nc"] = _build_program()
    return _PROGRAM_CACHE["nc"]


def kernel(left0, right0, left1, right1, left2, right2, _trace=False):
    from concourse.bass_utils import run_bass_kernel_spmd

    lefts = [left0, left1, left2]
    rights = [right0, right1, right2]
    nc = _get_program()

    idm = np.eye(128, dtype=np.float32)
    in_maps = []
    for k in range(N_CORES):
        m = {"idm": idm}
        for li, (W, H, D, Hc) in enumerate(LEVELS):
            Ls = np.asarray(lefts[li])[0, :, k * Hc:(k + 1) * Hc, :]
            Rs = np.asarray(rights[li])[0, :, k * Hc:(k + 1) * Hc, :]
            m[f"lp{li}"] = np.ascontiguousarray(
                np.pad(Ls, ((0, 0), (0, 0), (0, D))), dtype=np.float32)
            m[f"r{li}"] = np.ascontiguousarray(Rs, dtype=np.float32)
        in_maps.append(m)

    kw = {}
    if _trace:
        kw = dict(trace=True, trace_cores=[0])
    results = run_bass_kernel_spmd(nc, in_maps, core_ids=list(range(N_CORES)), **kw)

    out = []
    for li, (W, H, D, Hc) in enumerate(LEVELS):
        vol = np.zeros((1, D, H, W), dtype=np.float32)
        jj = np.arange(D)[:, None, None]
        xx = np.arange(W)[None, None, :]
        band = xx >= jj
        for k in range(N_CORES):
            dv = np.asarray(results.results[k][f"dev{li}"][:, :, :W], dtype=np.float32)
            vol[0, :, k * Hc:(k + 1) * Hc, :] = np.where(band, dv, 0.0)
        out.append(vol)
    if _trace:
        return tuple(out), results
    return tuple(out)


# revision 11
# speedup vs baseline: 2.3180x; 1.0201x over previous
"""Stereo cost-volume kernel for Trainium2 (8 NeuronCores, SPMD).

cost[n, j, h, x] = sum_c L[n,c,h,x] * R[n,c,h,x-j], zero when x < j.

Per core (h-sharded 1/8 slab of every pyramid level), groups of 8 blocks:
  - PE Gram tiles, K=C=32, bf16 inputs, fp32 PSUM (bank-aligned pairs):
      G[m, n] = sum_c R[c, b+m] * L[c, b+n]   (j = n - m >= 0 on free axis)
  - PSUM -> SBUF cast-copies to bf16, split 96/32 partitions (DVE + ACT)
  - group shear-DMA (diagonal AP, T=8 blocks/op): s2[p, t, j] = G_t[p, p+j]
  - PE transpose (identity matmul, bf16) -> pt[j, x'] PSUM
  - copy to SBUF octet tile, merged DMA-out (1KB runs) into right-padded
    bf16 DRAM dev[j, h, x]; host upcasts, slices pad, zero-fills x < j.
Emission is software-pipelined at group granularity.
"""
import numpy as np

# (W, H, D, Hc) per pyramid level; Hc = per-core h-slab with 8 cores
LEVELS = [(512, 256, 192, 32), (256, 128, 96, 16), (128, 64, 48, 8)]
N_CORES = 8
C = 32
T = 8  # blocks per shear/out group


def _build_program():
    import concourse.bacc as bacc
    import concourse.mybir as mybir
    import concourse.bass as bass
    from concourse.tile import TileContext

    f32 = mybir.dt.float32
    bf16 = mybir.dt.bfloat16

    nc = bacc.Bacc("TRN2", target_bir_lowering=False, debug=False)

    ins = {}
    outs = {}
    for li, (W, H, D, Hc) in enumerate(LEVELS):
        ins[f"lp{li}"] = nc.dram_tensor(f"lp{li}", [C, Hc, W + D], f32, kind="ExternalInput")
        ins[f"r{li}"] = nc.dram_tensor(f"r{li}", [C, Hc, W], f32, kind="ExternalInput")
        Wp = W + D - 1
        outs[f"dev{li}"] = nc.dram_tensor(f"dev{li}", [D, Hc, Wp], bf16, kind="ExternalOutput")
    idm = nc.dram_tensor("idm", [128, 128], f32, kind="ExternalInput")

    with TileContext(nc) as tc:
        with tc.tile_pool(name="const", bufs=1) as cpool:
            identb = cpool.tile([128, 128], bf16)
            nc.gpsimd.dma_start(identb[:], idm[:])  # SWDGE cast f32->bf16

            for li, (W, H, D, Hc) in enumerate(LEVELS):
                NW = 128 + D
                NA = 96 + D   # ga columns (rows 0..95 read cols [p, p+D))
                NB = 128 + D - 96  # gb columns (rows 96..127 read cols [96, 128+D))
                Wp = W + D - 1
                lp, r, dev = ins[f"lp{li}"], ins[f"r{li}"], outs[f"dev{li}"]
                devt = dev.ap().tensor

                blocks = [(h, b) for h in range(Hc) for b in range(0, W, 128)]
                assert len(blocks) % T == 0
                groups = [blocks[i:i + T] for i in range(0, len(blocks), T)]
                bpw = W // 128  # blocks per h-row
                jchunks = []
                j0 = 0
                while j0 < D:
                    jchunks.append((j0, min(96, D - j0)))
                    j0 += 96
                nch = len(jchunks)

                with (
                    tc.tile_pool(name=f"slab{li}", bufs=1) as spool,
                    tc.tile_pool(name=f"work{li}", bufs=4) as wpool,
                    tc.tile_pool(name=f"gpsum{li}", bufs=2, space="PSUM") as gpool,
                    tc.tile_pool(name=f"tpsum{li}", bufs=4, space="PSUM") as tpool,
                ):
                    lt = spool.tile([C, Hc, W + D], bf16)
                    rt = spool.tile([C, Hc, W], bf16)
                    nc.gpsimd.dma_start(lt[:], lp[:])   # SWDGE cast f32 -> bf16
                    nc.gpsimd.dma_start(rt[:], r[:])

                    state = {}

                    def st_mm(gi):
                        # 4 psum pair-tiles; interleave mm + drain copies
                        ga = wpool.tile([96, T, NA], bf16, tag="ga")
                        gb = wpool.tile([32, T, NB], bf16, tag="gb")
                        state[(gi, "gab")] = (ga, gb)
                        for pi in range(T // 2):
                            g = gpool.tile([128, 1024], f32, tag="g")
                            for t in range(2):
                                h, b = groups[gi][2 * pi + t]
                                nc.tensor.matmul(
                                    g[:, 512 * t:512 * t + NW],
                                    rt[:, h, b:b + 128],
                                    lt[:, h, b:b + NW],
                                    start=True, stop=True,
                                )
                            # drain this psum pair into group tiles
                            gsa = bass.AP(g.tensor, g.offset,
                                          [[g.ap[0][0], 128], [512, 2], [1, NA]])
                            gsb = bass.AP(g.tensor, g.offset + 96,
                                          [[g.ap[0][0], 128], [512, 2], [1, NB]])
                            nc.vector.tensor_copy(
                                ga[:, 2 * pi:2 * pi + 2, :], gsa[0:96])
                            nc.scalar.copy(
                                gb[:, 2 * pi:2 * pi + 2, :], gsb[96:128])

                    def st_shear(gi):
                        ga, gb = state.pop((gi, "gab"))
                        s2 = wpool.tile([128, T, D], bf16, tag="s2")
                        # rows 0..95: col p + j  (ga holds cols [0, NA))
                        nc.sync.dma_start(
                            bass.AP(s2.tensor, s2.offset,
                                    [[T * D, 96], [D, T], [1, D]]),
                            bass.AP(ga.tensor, ga.offset,
                                    [[T * NA + 1, 96], [NA, T], [1, D]]),
                        )
                        # rows 96..127: col p + j; gb holds cols [96, 128 + D)
                        nc.sync.dma_start(
                            bass.AP(s2.tensor, s2.offset + 96 * T * D,
                                    [[T * D, 32], [D, T], [1, D]]),
                            bass.AP(gb.tensor, gb.offset,
                                    [[T * NB + 1, 32], [NB, T], [1, D]]),
                        )
                        state[(gi, "s2")] = s2

                    def st_tr(gi):
                        s2 = state.pop((gi, "s2"))
                        st = wpool.tile([96, nch, T, 128], bf16, tag="st")
                        state[(gi, "st")] = st
                        for half in range(2):
                            pt = tpool.tile([96, nch, 4, 128], bf16, tag="pt")
                            for t4 in range(4):
                                t = half * 4 + t4
                                for ci, (j0, jn) in enumerate(jchunks):
                                    nc.tensor.transpose(
                                        pt[0:jn, ci, t4, :],
                                        s2[:, t, j0:j0 + jn],
                                        identb[:],
                                    )
                            dstap = bass.AP(
                                st.tensor, st.offset + half * 4 * 128,
                                [[nch * T * 128, 96], [T * 128, nch], [128, 4], [1, 128]])
                            if half == 0:
                                nc.vector.tensor_copy(dstap, pt[:])
                            else:
                                nc.scalar.copy(dstap, pt[:])

                    def st_out(gi):
                        dma_eng = nc.scalar if gi % 2 == 0 else nc.sync
                        st = state.pop((gi, "st"))
                        grp = groups[gi]
                        (h0, b0) = grp[0]
                        nrows = T // bpw  # h-rows in this group
                        for ci, (j0, jn) in enumerate(jchunks):
                            o0 = (j0 * Hc + h0) * Wp + b0 + j0
                            srcap = bass.AP(st.tensor, st.offset + ci * T * 128,
                                            [[nch * T * 128, jn], [1, T * 128]])
                            if nrows == 1:
                                dst = bass.AP(devt, o0,
                                              [[Hc * Wp + 1, jn], [1, T * 128]])
                            else:
                                dst = bass.AP(devt, o0,
                                              [[Hc * Wp + 1, jn], [Wp, nrows],
                                               [1, (T // nrows) * 128]])
                            dma_eng.dma_start(dst, srcap)

                    stages = [st_mm, st_shear, st_tr, st_out]
                    ns = len(stages)
                    ng = len(groups)
                    for step in range(ng + ns - 1):
                        for s in range(ns):
                            i = step - s
                            if 0 <= i < ng:
                                stages[s](i)
    nc.compile()
    return nc


_PROGRAM_CACHE = {}


def _get_program():
    if "nc" not in _PROGRAM_CACHE:
        _PROGRAM_CACHE["nc"] = _build_program()
    return _PROGRAM_CACHE["nc"]


def kernel(left0, right0, left1, right1, left2, right2, _trace=False):
    from concourse.bass_utils import run_bass_kernel_spmd

    lefts = [left0, left1, left2]
    rights = [right0, right1, right2]
    nc = _get_program()

    idm = np.eye(128, dtype=np.float32)
    in_maps = []
    for k in range(N_CORES):
        m = {"idm": idm}
        for li, (W, H, D, Hc) in enumerate(LEVELS):
            Ls = np.asarray(lefts[li])[0, :, k * Hc:(k + 1) * Hc, :]
            Rs = np.asarray(rights[li])[0, :, k * Hc:(k + 1) * Hc, :]
            m[f"lp{li}"] = np.ascontiguousarray(
                np.pad(Ls, ((0, 0), (0, 0), (0, D))), dtype=np.float32)
            m[f"r{li}"] = np.ascontiguousarray(Rs, dtype=np.float32)
        in_maps.append(m)

    kw = {}
    if _trace:
        kw = dict(trace=True, trace_cores=[0])
    results = run_bass_kernel_spmd(nc, in_maps, core_ids=list(range(N_CORES)), **kw)

    out = []
    for li, (W, H, D, Hc) in enumerate(LEVELS):
        vol = np.zeros((1, D, H, W), dtype=np.float32)
        jj = np.arange(D)[:, None, None]
        xx = np.arange(W)[None, None, :]
        band = xx >= jj
        for k in range(N_CORES):
            dv = np.asarray(results.results[k][f"dev{li}"][:, :, :W], dtype=np.float32)
            vol[0, :, k * Hc:(k + 1) * Hc, :] = np.where(band, dv, 0.0)
        out.append(vol)
    if _trace:
        return tuple(out), results
    return tuple(out)
